# revision 60
# baseline (speedup 1.0000x reference)
"""Trainium2 Bass kernel for nn_ActorNetwork (moe_routing).

Design (host-routed expert parallelism, zero collectives):
  reference semantics: with perm = stable argsort(idx),
    h_f[i] = relu(relu(state[perm[i]] @ W1[g(i)] + b1[g(i)]) @ W2 + b2)
    out[i] = tanh(h_f[i] @ W3[idx[i]] + b3[idx[i]])
  where g(i) (the W1 expert of sorted-position i) depends only on which
  sorted-count block position i falls into.  Core c takes exactly the sorted
  block of game c -> its layer-1 is ONE dense matmul with only W1[c].  Within
  the core, rows are sub-grouped by head game idx[i] so layer-3 is 8 dense
  per-group matmuls.  All routing (gather of state rows in, scatter of output
  rows back) happens on the host during shard/unshard.  Groups are stored
  sorted by size (descending) so the SPMD-uniform slot capacities
  M_j = max_core(j-th largest group) give ~1% padding.

  On-device layout is feature-major: activations live as [feature, row] so
  every matmul is lhsT=weight-tile [K=128, M=128], rhs=activation [K=128,
  N=rows], PSUM out [M features, rows].  Compute dtype bf16, f32 PSUM.
  All inputs are packed on the host into a handful of [128, X] images so each
  one moves with a single large DMA (per-DMA overhead ~0.6us dominates small
  transfers).  State is packed chunk-major so each 512-row chunk is one DMA.

Performance structure (measured at the 2.4GHz PE state):
  - L2's first two k-tiles (K=256 of 1024) run as ONE fp8 e4m3 DoubleRow
    matmul (2x PE rate): h1 m0-1 is written as e4m3 (x16) by scalar ACT,
    w2 k0-1 ships as an fp8 image (x4096), w2 k2-7 stays bf16 pre-scaled
    x2^16 so all PSUM partials share one scale, W3 is pre-scaled x2^-16.
    Costs sqrt(2/8)*3.6% ~= 1.8% quantization error (2e-2 budget), saves
    8N PE column-cycles (~3.7us).
  - PE stream is gapless (<0.5us of stalls): relu work alternates between
    vector (tensor_scalar) and scalar (ACT Relu) by w1-image position
    parity; a 6-deep PSUM pool absorbs wave-boundary handoffs; L1 waves
    run chunk-outer (c0 m0-3, c0 m4-7, c1 ...) tracking DMA arrival.
  - DMA descriptor order is latency-tuned: the first L1 wave's pieces
    (st_c0|w1 "a" halves) lead both HWDGE rings, bias early, w2/w3 last;
    ~74 zero-data warmup matmuls ramp the PE p-state (needs ~3us busy)
    while the first pieces land (~11.3us: boot 7.2 + desc 0.7 + transfer
    + ~2us DMA completion latency).
  - Tail ~4.8us is floor: last tanh ACT + out-DMA desc-gen + ~2us DMA
    completion + NEFF epilogue.  Total ~62.2us vs 47.2us pure-matmul
    floor at bf16 (68.9us before this tuning round).
  NOTE: the shared trn2 pool's PE clock wanders between ~2.4GHz (216ns
  per 512-col matmul) and ~2.0GHz (259ns) run to run; absolute times
  scale by ~1.2x with it.
"""

import numpy as np
import ml_dtypes

_BF16 = ml_dtypes.bfloat16
_FP8 = ml_dtypes.float8_e4m3     # TRN fp8e4: IEEE-ish e4m3, max normal 240
_NCORES = 8
_SH = 16.0                       # h1 fp8 scale (m0-1)
_SW0 = 256.0                     # w2 k0-1 fp8 scale (16*256 = 2^12)
_SW1 = 4096.0                    # w2 k2-5 fp8 scale (h1q2 at scale 1)
_SF = _SH * _SW0                 # 2^12: scale of all L2 partials / hf
# w1 image position -> logical m.  Relu engine alternates by position
# parity (even -> vector, odd -> scalar) so every L1 wave splits its 4
# relus across both engines; logical m0-1 (fp8, scalar ACT) sit at odd
# positions.
_MPERM = (2, 0, 3, 1, 4, 6, 5, 7)
_WARM_MMS = 74  # PE warmup matmuls (~1us cold) to ramp HAM before real work
_graph_cache: dict = {}


def _make_plan(idx: np.ndarray, G: int):
    """Host routing plan: which (sorted-position) rows go to which core/slot."""
    idx = np.asarray(idx)
    perm = np.argsort(idx, kind="stable")
    counts = np.bincount(idx, minlength=G)
    cum = np.zeros(G + 1, dtype=np.int64)
    cum[1:] = np.cumsum(counts)

    core_groups = []  # per core: list of (head_game, sorted_positions) desc by size
    for c in range(G):
        pos = np.arange(cum[c], cum[c + 1])
        heads = idx[pos]
        groups = [(b, pos[heads == b]) for b in range(G)]
        groups.sort(key=lambda t: (-len(t[1]), t[0]))
        core_groups.append(groups)

    sizes = np.array([[len(p) for _, p in groups] for groups in core_groups])
    M = sizes.max(axis=0)          # slot capacity per position (SPMD-uniform)
    keep = M > 0
    M = M[keep]
    core_groups = [[g for g, k in zip(groups, keep) if k] for groups in core_groups]
    NG = len(M)
    starts = np.zeros(NG + 1, dtype=np.int64)
    starts[1:] = np.cumsum(M)
    N = int(starts[-1])
    return perm, core_groups, M, starts, N


def _chunks_of(N):
    # 512-wide chunks: matmuls with 512 moving cols fully hide LDWEIGHTS
    # (~190ns observed at 365 cols vs 213ns at 512 = 25% LDW exposure).
    out = []
    c0 = 0
    while c0 < N:
        cw = min(512, N - c0)
        out.append((c0, cw))
        c0 += cw
    return out


def _build_graph(D, H1, H2, A, NG, starts, N):
    """Build + finalize the SPMD Bass graph (identical for all cores)."""
    from concourse import bacc
    import concourse.mybir as mybir
    from concourse.tile import TileContext

    bf = mybir.dt.bfloat16
    f32 = mybir.dt.float32
    fp8 = mybir.dt.float8e4
    KD, K1, K2 = D // 128, H1 // 128, H2 // 128
    M1, M2 = H1 // 128, H2 // 128
    assert D % 128 == 0 and H1 % 128 == 0 and H2 % 128 == 0 and A == 128

    chunks = _chunks_of(N)
    # One fused bf16 input image per core (single SBUF tile, range-tracked):
    #   [ st_c0_k0 | w1_k0 | ... | st_c0_k3 | w1_k3 | st_c1.. | w2 k2-7 | w3 ]
    # so the critical first-wave pieces stream pairwise on the fast qACT
    # queue with few dma_start instructions (desc-gen ~0.65us each).
    # L2's k0-1 contraction runs in fp8 e4m3 DoubleRow (one matmul covers
    # both k-tiles): h1 m0-1 is stored as fp8 (x16), w2 k0-1 ships as a
    # separate fp8 image (x4096), the bf16 w2 k2-7 is pre-scaled x2^16 so
    # all PSUM partials share one scale, and W3 is pre-scaled x2^-16 to
    # undo it.  Costs ~sqrt(2/8)*3.6% quantization error (total ~1.8%,
    # budget 2%), saves 8N PE column-cycles (~3.7us).
    c0w0 = chunks[0][1]
    PAIR = c0w0 + H1
    P0 = KD * PAIR                      # end of (st_c0 | w1) pairs
    P1 = P0 + KD * (N - c0w0)           # end of st_c1.. ; start of w2 k2-7
    P2 = P1 + (K1 - 2) * H2             # start of w3
    IMG_W = P2 + NG * K2 * A

    def st_col(k, c0, cw):
        if c0 == 0:
            return k * PAIR
        return P0 + (c0 - c0w0) * KD + k * cw

    def w1_col(k):
        return k * PAIR + c0w0

    nc = bacc.Bacc("TRN2")
    img_ext = nc.declare_dram_parameter("img", [128, IMG_W], bf, isOutput=False)
    w2q_ext = nc.declare_dram_parameter("w2q", [128, 6 * H2], fp8, isOutput=False)
    b_ext = nc.declare_dram_parameter("bs", [128, M1 + M2 + NG], f32, isOutput=False)
    out_ext = nc.declare_dram_parameter("out", [A, N], f32, isOutput=True)

    add = mybir.AluOpType.add
    amax = mybir.AluOpType.max
    Tanh = mybir.ActivationFunctionType.Tanh
    Relu = mybir.ActivationFunctionType.Relu

    with TileContext(nc) as tc:
        with (
            tc.tile_pool(name="weights", bufs=1) as wp,
            tc.tile_pool(name="acts", bufs=1) as ap,
            tc.tile_pool(name="psum", bufs=6, space="PSUM") as pp,
            tc.tile_pool(name="psum3", bufs=2, space="PSUM") as pp3,
        ):
            img = wp.tile([128, IMG_W], bf, name="img", tag="img")
            w2qt = wp.tile([128, 6, H2], fp8, name="w2qt", tag="w2qt")
            bt = wp.tile([128, M1 + M2 + NG], f32, name="bt", tag="bt")
            h1q = ap.tile([128, 2, N], fp8, name="h1q", tag="h1q")
            c2w = chunks[-1][1] if chunks[-1][1] < 512 else 0
            # the small tail chunk runs L2 k0-5 in fp8 (3 DoubleRow matmuls):
            # its h1 m2-5 lives here as e4m3 at scale 1 (vector relu writes it
            # with the plain bias; w2 k2-5 carries the full 2^12)
            h1q2 = ap.tile([128, 4, c2w], fp8, name="h1q2", tag="h1q2") if c2w else None
            h1 = [ap.tile([128, N], bf, name=f"h1_{m}", tag=f"h1_{m}") if m >= 2 else None
                  for m in range(M1)]
            hf = [ap.tile([128, N], bf, name=f"hf_{m}", tag=f"hf_{m}") for m in range(M2)]
            osb = ap.tile([A, N], f32, name="osb", tag="osb")
            wrm = wp.tile([128, 64], bf, name="wrm", tag="wrm")

            # PE warmup: ramp the HAM clock gate to 8/8 while DMAs land.
            nc.gpsimd.memset(wrm[:], 0)
            pw = pp3.tile([64, 64], f32, name="psw", tag="ps3")
            for _ in range(_WARM_MMS):
                nc.tensor.matmul(pw[:], wrm[:, :64], wrm[:], start=True, stop=True)

            # DMAs: descriptor-gen is ~0.6us per dma_start per queue and the
            # two HWDGE rings share the ~436GB/s AXI ports, so ORDER is what
            # matters: land the first L1 wave's data (st_c0|w1 m0-3 "a"
            # halves of every k) before any "b" half, bias early, then state
            # c1.., then w2 (scalar ring) / w3 (sync ring) behind everything.
            def amid(k):
                return k * PAIR + c0w0 + H1 // 2

            for k in range(KD):                                             # a halves
                eng = nc.sync if k % 2 == 0 else nc.scalar
                eng.dma_start(img[:, k * PAIR : amid(k)], img_ext[:, k * PAIR : amid(k)])
            nc.sync.dma_start(bt[:], b_ext[:])
            stm = P0 + (P1 - P0) // 2
            if P1 > P0:
                nc.scalar.dma_start(img[:, P0:stm], img_ext[:, P0:stm])    # st_c1 k0-1
            # b halves; scalar's desc burst must end early (it also runs the
            # odd-position relu1 ACTs from ~13.5us), so sync carries most of
            # the mid-priority pieces
            nc.sync.dma_start(img[:, amid(0) : PAIR], img_ext[:, amid(0) : PAIR])
            nc.scalar.dma_start(img[:, amid(1) : 2 * PAIR], img_ext[:, amid(1) : 2 * PAIR])
            nc.sync.dma_start(img[:, amid(2) : 3 * PAIR], img_ext[:, amid(2) : 3 * PAIR])
            if P1 > stm:
                nc.sync.dma_start(img[:, stm:P1], img_ext[:, stm:P1])      # st_c1 k2-3, c2
            nc.scalar.dma_start(img[:, amid(3) : 4 * PAIR], img_ext[:, amid(3) : 4 * PAIR])
            nc.sync.dma_start(w2qt[:, :, :], w2q_ext[:])                   # w2 k0-1 fp8
            half2 = P1 + ((K1 - 2) // 2) * H2
            nc.scalar.dma_start(img[:, P1:half2], img_ext[:, P1:half2])    # w2 k2-4
            nc.scalar.dma_start(img[:, half2:P2], img_ext[:, half2:P2])    # w2 k5-7
            nc.sync.dma_start(img[:, P2:IMG_W], img_ext[:, P2:IMG_W])      # w3

            def l1_mm(ps, m, k, c0, cw):
                w0 = w1_col(k) + m * 128
                s0 = st_col(k, c0, cw)
                nc.tensor.matmul(
                    ps[:],
                    img[:, w0 : w0 + 128],
                    img[:, s0 : s0 + cw],
                    start=(k == 0),
                    stop=(k == KD - 1),
                )

            DR = mybir.MatmulPerfMode.DoubleRow

            def l2_dr(ps, m, kk, rhs, start):
                # fp8 DoubleRow: one matmul contracts k-tiles kk and kk+1
                nc.tensor.matmul(
                    ps[:],
                    w2qt[:, kk : kk + 2, m * 128 : (m + 1) * 128],
                    rhs,
                    start=start,
                    stop=False,
                    perf_mode=DR,
                    skip_group_check=True,
                )

            def l2_mm(ps, m, k, sl):
                w0 = P1 + (k - 2) * H2 + m * 128
                nc.tensor.matmul(
                    ps[:],
                    img[:, w0 : w0 + 128],
                    h1[k][:, sl],
                    start=False,
                    stop=(k == K1 - 1),
                    skip_group_check=True,
                )

            # relu split: image positions 0-3 (first wave) on vector,
            # positions 4-7 (second wave) on scalar ACT — halves the
            # elementwise backlog that would block PSUM-pool recycling.
            # Logical m0-1 are fp8: out = Relu(16*ps + 16*b1) cast to e4m3
            # (bias pre-scaled host-side).
            def relu1(ps, p, sl, in_c2=False):
                lm = _MPERM[p]
                if lm < 2:
                    nc.scalar.activation(
                        h1q[:, lm, sl], ps[:], Relu, bias=bt[:, p : p + 1], scale=16.0
                    )
                elif in_c2 and lm < 6:
                    # tail chunk: write scale-1 e4m3 for the extra fp8 k-pairs
                    # (same op/engine/bias as the bf16 path, fp8 output dtype)
                    nc.vector.tensor_scalar(
                        h1q2[:, lm - 2, :], ps[:], bt[:, p : p + 1], 0.0, add, amax
                    )
                elif p % 2 == 0:
                    nc.vector.tensor_scalar(
                        h1[lm][:, sl], ps[:], bt[:, p : p + 1], 0.0, add, amax
                    )
                else:
                    nc.scalar.activation(
                        h1[lm][:, sl], ps[:], Relu, bias=bt[:, p : p + 1]
                    )

            def relu2(ps, m, sl):
                if m < M2 // 2:
                    nc.vector.tensor_scalar(
                        hf[m][:, sl], ps[:], bt[:, M1 + m : M1 + m + 1], 0.0, add, amax
                    )
                else:
                    nc.scalar.activation(
                        hf[m][:, sl], ps[:], Relu, bias=bt[:, M1 + m : M1 + m + 1]
                    )

            # Phase 1: L1 for ALL chunks (needs only st+w1 ~2.1MB) so w2/w3
            # stream in behind the compute.  Wave order tracks DMA arrival:
            # (m0-3, c0) needs only the "a" halves, (m0-3, c1) adds st_c1,
            # (m4-7, *) needs the "b" halves which land meanwhile.  The
            # small tail chunk (64 cols) runs last, per (m, k).
            big = [c for c in chunks if c[1] == 512]
            small = [c for c in chunks if c[1] < 512]
            for cc0, ccw in big:
                for half in (range(0, 4), range(4, M1)):
                    pss = [pp.tile([128, ccw], f32, name="ps", tag="ps") for _ in half]
                    for k in range(KD):
                        for i, m in enumerate(half):
                            l1_mm(pss[i], m, k, cc0, ccw)
                    for i, m in enumerate(half):
                        relu1(pss[i], m, slice(cc0, cc0 + ccw))
            for m in range(M1):
                for cc0, ccw in small:
                    ps = pp.tile([128, ccw], f32, name="ps", tag="ps")
                    for k in range(KD):
                        l1_mm(ps, m, k, cc0, ccw)
                    relu1(ps, m, slice(cc0, cc0 + ccw), in_c2=True)
            # Phase 2+3: L2 per chunk, then L3 for fully-covered groups with
            # one batched out-DMA per chunk.
            done_j = 0
            for ci, (c0, cw) in enumerate(chunks):
                sl = slice(c0, c0 + cw)
                for m in range(M2):
                    ps = pp.tile([128, cw], f32, name="ps", tag="ps")
                    l2_dr(ps, m, 0, h1q[:, :, sl], True)
                    if cw < 512 and c2w:
                        l2_dr(ps, m, 2, h1q2[:, 0:2, :], False)
                        l2_dr(ps, m, 4, h1q2[:, 2:4, :], False)
                        ks = range(6, K1)
                    else:
                        ks = range(2, K1)
                    for k in ks:
                        l2_mm(ps, m, k, sl)
                    relu2(ps, m, sl)
                lim = c0 + cw
                while done_j < NG and starts[done_j + 1] <= lim:
                    j = done_j
                    sj, ej = int(starts[j]), int(starts[j + 1])
                    # split groups wider than one PSUM bank (512 f32)
                    for g0 in range(sj, ej, 512):
                        g1 = min(g0 + 512, ej)
                        ps = pp3.tile([A, g1 - g0], f32, name="ps3", tag="ps3")
                        for k in range(K2):
                            w0 = P2 + j * K2 * A + k * A
                            nc.tensor.matmul(
                                ps[:],
                                img[:, w0 : w0 + A],
                                hf[k][:, g0:g1],
                                start=(k == 0),
                                stop=(k == K2 - 1),
                            )
                        nc.scalar.activation(
                            osb[:, g0:g1], ps[:], Tanh,
                            bias=bt[:, M1 + M2 + j : M1 + M2 + j + 1],
                        )
                        nc.sync.dma_start(out_ext[:, g0:g1], osb[:, g0:g1])
                    done_j += 1
            assert done_j == NG

    nc.finalize()
    return nc


def _kmajor(w, K):
    """[K*128, F] -> [128, K*F] with col = k*F + f."""
    F = w.shape[1]
    return np.ascontiguousarray(w.reshape(K, 128, F).transpose(1, 0, 2).reshape(128, K * F))


def _prepare(state, idx, W1, b1, W2, b2, W3, b3):
    state = np.ascontiguousarray(np.asarray(state, dtype=np.float32))
    idx = np.asarray(idx)
    W1 = np.asarray(W1, dtype=np.float32)
    b1 = np.asarray(b1, dtype=np.float32)
    W2 = np.asarray(W2, dtype=np.float32)
    b2 = np.asarray(b2, dtype=np.float32)
    W3 = np.asarray(W3, dtype=np.float32)
    b3 = np.asarray(b3, dtype=np.float32)

    B, D = state.shape
    G, _, H1 = W1.shape
    H2 = W2.shape[1]
    A = W3.shape[2]
    KD, K1, K2 = D // 128, H1 // 128, H2 // 128
    M1, M2 = H1 // 128, H2 // 128
    assert G == _NCORES, f"expert-parallel plan assumes {_NCORES} games, got {G}"

    perm, core_groups, M, starts, N = _make_plan(idx, G)
    NG = len(M)
    chunks = _chunks_of(N)

    key = (D, H1, H2, A, NG, tuple(int(x) for x in starts), N)
    if key not in _graph_cache:
        _graph_cache[key] = _build_graph(D, H1, H2, A, NG, starts, N)
    nc = _graph_cache[key]

    # fused image column offsets (must mirror _build_graph)
    c0w0 = chunks[0][1]
    PAIR = c0w0 + H1
    P0 = KD * PAIR
    P1 = P0 + KD * (N - c0w0)
    P2 = P1 + (K1 - 2) * H2
    IMG_W = P2 + NG * K2 * A

    # L2 k0-1 in fp8 (x4096), k2-7 in bf16 (x2^16); b2 carries the 2^16,
    # W3 divides it back out.  All scales are powers of two (exact).
    w2q_f = np.concatenate([W2[:256] * _SW0, W2[256:768] * _SW1])
    w2q = np.clip(w2q_f, -240.0, 240.0).astype(_FP8)
    w2q = np.ascontiguousarray(
        w2q.reshape(6, 128, H2).transpose(1, 0, 2).reshape(128, 6 * H2)
    )
    w2_h = _kmajor((W2[256:] * _SF).astype(_BF16), K1 - 2)
    b2_col = (b2 * _SF).reshape(M2, 128).T.astype(np.float32)

    in_maps = []
    scatters = []  # per core: list of (sorted_positions, col_start)
    for c in range(G):
        sT = np.zeros((D, N), dtype=_BF16)
        img = np.zeros((128, IMG_W), dtype=_BF16)
        bs = np.zeros((128, M1 + M2 + NG), dtype=np.float32)
        bs[:, :M1] = b1[c].reshape(M1, 128)[list(_MPERM)].T
        for p_, lm_ in enumerate(_MPERM):   # relu1 logical m0-1: Relu(16*ps+16*b1)
            if lm_ < 2:
                bs[:, p_] *= _SH
        bs[:, M1 : M1 + M2] = b2_col
        sc = []
        for j, (head, pos) in enumerate(core_groups[c]):
            s0 = int(starts[j])
            if len(pos):
                sT[:, s0 : s0 + len(pos)] = state[perm[pos]].T.astype(_BF16)
                sc.append((pos, s0))
            img[:, P2 + j * K2 * A : P2 + (j + 1) * K2 * A] = (
                (W3[head] / _SF).astype(_BF16)
                .reshape(K2, 128, A).transpose(1, 0, 2).reshape(128, K2 * A)
            )
            bs[:, M1 + M2 + j] = b3[head]
        w1p = W1[c].reshape(D, M1, 128)[:, list(_MPERM), :].reshape(D, H1)
        w1_h = _kmajor(w1p.astype(_BF16), KD)    # [128, KD*H1], col k*H1+m
        for k in range(KD):
            img[:, k * PAIR : k * PAIR + c0w0] = sT[k * 128 : (k + 1) * 128, 0:c0w0]
            img[:, k * PAIR + c0w0 : (k + 1) * PAIR] = w1_h[:, k * H1 : (k + 1) * H1]
        for c0, cw in chunks[1:]:
            img[:, P0 + (c0 - c0w0) * KD : P0 + (c0 - c0w0 + cw) * KD] = (
                sT[:, c0 : c0 + cw].reshape(KD, 128, cw).transpose(1, 0, 2).reshape(128, KD * cw)
            )
        img[:, P1:P2] = w2_h
        in_maps.append({"img": img, "bs": bs, "w2q": w2q})
        scatters.append(sc)
    return nc, in_maps, scatters, (B, A)


def _run(state, idx, W1, b1, W2, b2, W3, b3, trace=False, trace_kwargs=None):
    from concourse.bass_utils import run_bass_kernel_spmd

    nc, in_maps, scatters, (B, A) = _prepare(state, idx, W1, b1, W2, b2, W3, b3)
    res = run_bass_kernel_spmd(
        nc,
        in_maps,
        core_ids=list(range(_NCORES)),
        trace=trace,
        **(trace_kwargs or {}),
    )
    out = np.zeros((B, A), dtype=np.float32)
    for c in range(len(scatters)):
        o = np.asarray(res.results[c]["out"], dtype=np.float32)  # [A, N]
        for pos, s0 in scatters[c]:
            out[pos] = o[:, s0 : s0 + len(pos)].T
    return out, res


def kernel(**inputs) -> np.ndarray:
    out, _ = _run(**inputs)
    return out



# revision 64
# speedup vs baseline: 1.0054x; 1.0054x over previous
"""Trainium2 Bass kernel for nn_ActorNetwork (moe_routing).

Design (host-routed expert parallelism, zero collectives):
  reference semantics: with perm = stable argsort(idx),
    h_f[i] = relu(relu(state[perm[i]] @ W1[g(i)] + b1[g(i)]) @ W2 + b2)
    out[i] = tanh(h_f[i] @ W3[idx[i]] + b3[idx[i]])
  where g(i) (the W1 expert of sorted-position i) depends only on which
  sorted-count block position i falls into.  Core c takes exactly the sorted
  block of game c -> its layer-1 is ONE dense matmul with only W1[c].  Within
  the core, rows are sub-grouped by head game idx[i] so layer-3 is 8 dense
  per-group matmuls.  All routing (gather of state rows in, scatter of output
  rows back) happens on the host during shard/unshard.  Groups are stored
  sorted by size (descending) so the SPMD-uniform slot capacities
  M_j = max_core(j-th largest group) give ~1% padding.

  On-device layout is feature-major: activations live as [feature, row] so
  every matmul is lhsT=weight-tile [K=128, M=128], rhs=activation [K=128,
  N=rows], PSUM out [M features, rows].  Compute dtype bf16, f32 PSUM.
  All inputs are packed on the host into a handful of [128, X] images so each
  one moves with a single large DMA (per-DMA overhead ~0.6us dominates small
  transfers).  State is packed chunk-major so each 512-row chunk is one DMA.

Performance structure (measured at the 2.4GHz PE state):
  - L2's first two k-tiles (K=256 of 1024) run as ONE fp8 e4m3 DoubleRow
    matmul (2x PE rate): h1 m0-1 is written as e4m3 (x16) by scalar ACT,
    w2 k0-1 ships as an fp8 image (x4096), w2 k2-7 stays bf16 pre-scaled
    x2^16 so all PSUM partials share one scale, W3 is pre-scaled x2^-16.
    Costs sqrt(2/8)*3.6% ~= 1.8% quantization error (2e-2 budget), saves
    8N PE column-cycles (~3.7us).
  - PE stream is gapless (<0.5us of stalls): relu work alternates between
    vector (tensor_scalar) and scalar (ACT Relu) by w1-image position
    parity; a 6-deep PSUM pool absorbs wave-boundary handoffs; L1 waves
    run chunk-outer (c0 m0-3, c0 m4-7, c1 ...) tracking DMA arrival.
  - DMA descriptor order is latency-tuned: the first L1 wave's pieces
    (st_c0|w1 "a" halves) lead both HWDGE rings, bias early, w2/w3 last;
    ~74 zero-data warmup matmuls ramp the PE p-state (needs ~3us busy)
    while the first pieces land (~11.3us: boot 7.2 + desc 0.7 + transfer
    + ~2us DMA completion latency).
  - Tail ~4.8us is floor: last tanh ACT + out-DMA desc-gen + ~2us DMA
    completion + NEFF epilogue.  Total ~62.2us vs 47.2us pure-matmul
    floor at bf16 (68.9us before this tuning round).
  NOTE: the shared trn2 pool's PE clock wanders between ~2.4GHz (216ns
  per 512-col matmul) and ~2.0GHz (259ns) run to run; absolute times
  scale by ~1.2x with it.
"""

import numpy as np
import ml_dtypes

_BF16 = ml_dtypes.bfloat16
_FP8 = ml_dtypes.float8_e4m3     # TRN fp8e4: IEEE-ish e4m3, max normal 240
_NCORES = 8
_SH = 16.0                       # h1 fp8 scale (m0-1)
_SW0 = 256.0                     # w2 k0-1 fp8 scale (16*256 = 2^12)
_SW1 = 4096.0                    # w2 k2-5 fp8 scale (h1q2 at scale 1)
_SF = _SH * _SW0                 # 2^12: scale of all L2 partials / hf
# w1 image position -> logical m.  Relu engine alternates by position
# parity (even -> vector, odd -> scalar) so every L1 wave splits its 4
# relus across both engines; logical m0-1 (fp8, scalar ACT) sit at odd
# positions.
_MPERM = (2, 0, 3, 1, 4, 6, 5, 7)
_WARM_MMS = 74  # PE warmup matmuls (~1us cold) to ramp HAM before real work
_graph_cache: dict = {}


def _make_plan(idx: np.ndarray, G: int):
    """Host routing plan: which (sorted-position) rows go to which core/slot."""
    idx = np.asarray(idx)
    perm = np.argsort(idx, kind="stable")
    counts = np.bincount(idx, minlength=G)
    cum = np.zeros(G + 1, dtype=np.int64)
    cum[1:] = np.cumsum(counts)

    core_groups = []  # per core: list of (head_game, sorted_positions) desc by size
    for c in range(G):
        pos = np.arange(cum[c], cum[c + 1])
        heads = idx[pos]
        groups = [(b, pos[heads == b]) for b in range(G)]
        groups.sort(key=lambda t: (-len(t[1]), t[0]))
        core_groups.append(groups)

    sizes = np.array([[len(p) for _, p in groups] for groups in core_groups])
    M = sizes.max(axis=0)          # slot capacity per position (SPMD-uniform)
    keep = M > 0
    M = M[keep]
    core_groups = [[g for g, k in zip(groups, keep) if k] for groups in core_groups]
    NG = len(M)
    starts = np.zeros(NG + 1, dtype=np.int64)
    starts[1:] = np.cumsum(M)
    N = int(starts[-1])
    return perm, core_groups, M, starts, N


def _chunks_of(N):
    # 512-wide chunks: matmuls with 512 moving cols fully hide LDWEIGHTS
    # (~190ns observed at 365 cols vs 213ns at 512 = 25% LDW exposure).
    out = []
    c0 = 0
    while c0 < N:
        cw = min(512, N - c0)
        out.append((c0, cw))
        c0 += cw
    return out


def _build_graph(D, H1, H2, A, NG, starts, N):
    """Build + finalize the SPMD Bass graph (identical for all cores)."""
    from concourse import bacc
    import concourse.mybir as mybir
    from concourse.tile import TileContext

    bf = mybir.dt.bfloat16
    f32 = mybir.dt.float32
    fp8 = mybir.dt.float8e4
    KD, K1, K2 = D // 128, H1 // 128, H2 // 128
    M1, M2 = H1 // 128, H2 // 128
    assert D % 128 == 0 and H1 % 128 == 0 and H2 % 128 == 0 and A == 128

    chunks = _chunks_of(N)
    # One fused bf16 input image per core (single SBUF tile, range-tracked):
    #   [ st_c0_k0 | w1_k0 | ... | st_c0_k3 | w1_k3 | st_c1.. | w2 k2-7 | w3 ]
    # so the critical first-wave pieces stream pairwise on the fast qACT
    # queue with few dma_start instructions (desc-gen ~0.65us each).
    # L2's k0-1 contraction runs in fp8 e4m3 DoubleRow (one matmul covers
    # both k-tiles): h1 m0-1 is stored as fp8 (x16), w2 k0-1 ships as a
    # separate fp8 image (x4096), the bf16 w2 k2-7 is pre-scaled x2^16 so
    # all PSUM partials share one scale, and W3 is pre-scaled x2^-16 to
    # undo it.  Costs ~sqrt(2/8)*3.6% quantization error (total ~1.8%,
    # budget 2%), saves 8N PE column-cycles (~3.7us).
    c0w0 = chunks[0][1]
    PAIR = c0w0 + H1
    P0 = KD * PAIR                      # end of (st_c0 | w1) pairs
    P1 = P0 + KD * (N - c0w0)           # end of st_c1.. ; start of w2 k2-7
    P2 = P1 + (K1 - 2) * H2             # start of w3
    IMG_W = P2 + NG * K2 * A

    def st_col(k, c0, cw):
        if c0 == 0:
            return k * PAIR
        return P0 + (c0 - c0w0) * KD + k * cw

    def w1_col(k):
        return k * PAIR + c0w0

    nc = bacc.Bacc("TRN2")
    img_ext = nc.declare_dram_parameter("img", [128, IMG_W], bf, isOutput=False)
    w2q_ext = nc.declare_dram_parameter("w2q", [128, 6 * H2], fp8, isOutput=False)
    b_ext = nc.declare_dram_parameter("bs", [128, M1 + M2 + NG], f32, isOutput=False)
    out_ext = nc.declare_dram_parameter("out", [A, N], f32, isOutput=True)

    add = mybir.AluOpType.add
    amax = mybir.AluOpType.max
    Tanh = mybir.ActivationFunctionType.Tanh
    Relu = mybir.ActivationFunctionType.Relu

    with TileContext(nc) as tc:
        with (
            tc.tile_pool(name="weights", bufs=1) as wp,
            tc.tile_pool(name="acts", bufs=1) as ap,
            tc.tile_pool(name="psum", bufs=6, space="PSUM") as pp,
            tc.tile_pool(name="psum3", bufs=2, space="PSUM") as pp3,
        ):
            img = wp.tile([128, IMG_W], bf, name="img", tag="img")
            w2qt = wp.tile([128, 6, H2], fp8, name="w2qt", tag="w2qt")
            bt = wp.tile([128, M1 + M2 + NG], f32, name="bt", tag="bt")
            h1q = ap.tile([128, 2, N], fp8, name="h1q", tag="h1q")
            c2w = chunks[-1][1] if chunks[-1][1] < 512 else 0
            # the small tail chunk runs L2 k0-5 in fp8 (3 DoubleRow matmuls):
            # its h1 m2-5 lives here as e4m3 at scale 1 (vector relu writes it
            # with the plain bias; w2 k2-5 carries the full 2^12)
            h1q2 = ap.tile([128, 4, c2w], fp8, name="h1q2", tag="h1q2") if c2w else None
            h1 = [ap.tile([128, N], bf, name=f"h1_{m}", tag=f"h1_{m}") if m >= 2 else None
                  for m in range(M1)]
            hf = [ap.tile([128, N], bf, name=f"hf_{m}", tag=f"hf_{m}") for m in range(M2)]
            osb = ap.tile([A, N], f32, name="osb", tag="osb")
            wrm = wp.tile([128, 64], bf, name="wrm", tag="wrm")

            # PE warmup: ramp the HAM clock gate to 8/8 while DMAs land.
            nc.gpsimd.memset(wrm[:], 0)
            pw = pp3.tile([64, 64], f32, name="psw", tag="ps3")
            for _ in range(_WARM_MMS):
                nc.tensor.matmul(pw[:], wrm[:, :64], wrm[:], start=True, stop=True)

            # DMAs: descriptor-gen is ~0.6us per dma_start per queue and the
            # two HWDGE rings share the ~436GB/s AXI ports, so ORDER is what
            # matters: land the first L1 wave's data (st_c0|w1 m0-3 "a"
            # halves of every k) before any "b" half, bias early, then state
            # c1.., then w2 (scalar ring) / w3 (sync ring) behind everything.
            def amid(k):
                return k * PAIR + c0w0 + H1 // 2

            for k in range(KD):                                             # a halves
                eng = nc.sync if k % 2 == 0 else nc.scalar
                eng.dma_start(img[:, k * PAIR : amid(k)], img_ext[:, k * PAIR : amid(k)])
            nc.sync.dma_start(bt[:], b_ext[:])
            stm = P0 + (P1 - P0) // 2
            if P1 > P0:
                nc.scalar.dma_start(img[:, P0:stm], img_ext[:, P0:stm])    # st_c1 k0-1
            # b halves; scalar's desc burst must end early (it also runs the
            # odd-position relu1 ACTs from ~13.5us), so sync carries most of
            # the mid-priority pieces
            nc.sync.dma_start(img[:, amid(0) : PAIR], img_ext[:, amid(0) : PAIR])
            nc.scalar.dma_start(img[:, amid(1) : 2 * PAIR], img_ext[:, amid(1) : 2 * PAIR])
            nc.sync.dma_start(img[:, amid(2) : 3 * PAIR], img_ext[:, amid(2) : 3 * PAIR])
            if P1 > stm:
                nc.sync.dma_start(img[:, stm:P1], img_ext[:, stm:P1])      # st_c1 k2-3, c2
            nc.scalar.dma_start(img[:, amid(3) : 4 * PAIR], img_ext[:, amid(3) : 4 * PAIR])
            nc.sync.dma_start(w2qt[:, :, :], w2q_ext[:])                   # w2 k0-1 fp8
            half2 = P1 + ((K1 - 2) // 2) * H2
            nc.scalar.dma_start(img[:, P1:half2], img_ext[:, P1:half2])    # w2 k2-4
            nc.scalar.dma_start(img[:, half2:P2], img_ext[:, half2:P2])    # w2 k5-7
            nc.sync.dma_start(img[:, P2:IMG_W], img_ext[:, P2:IMG_W])      # w3

            def l1_mm(ps, m, k, c0, cw):
                w0 = w1_col(k) + m * 128
                s0 = st_col(k, c0, cw)
                nc.tensor.matmul(
                    ps[:],
                    img[:, w0 : w0 + 128],
                    img[:, s0 : s0 + cw],
                    start=(k == 0),
                    stop=(k == KD - 1),
                )

            DR = mybir.MatmulPerfMode.DoubleRow

            def l2_dr(ps, m, kk, rhs, start, stop=False):
                # fp8 DoubleRow: one matmul contracts k-tiles kk and kk+1
                nc.tensor.matmul(
                    ps[:],
                    w2qt[:, kk : kk + 2, m * 128 : (m + 1) * 128],
                    rhs,
                    start=start,
                    stop=stop,
                    perf_mode=DR,
                    skip_group_check=True,
                )

            def l2_mm(ps, m, k, sl, stop=None):
                w0 = P1 + (k - 2) * H2 + m * 128
                nc.tensor.matmul(
                    ps[:],
                    img[:, w0 : w0 + 128],
                    h1[k][:, sl],
                    start=False,
                    stop=(k == K1 - 1) if stop is None else stop,
                    skip_group_check=True,
                )

            # relu split: image positions 0-3 (first wave) on vector,
            # positions 4-7 (second wave) on scalar ACT — halves the
            # elementwise backlog that would block PSUM-pool recycling.
            # Logical m0-1 are fp8: out = Relu(16*ps + 16*b1) cast to e4m3
            # (bias pre-scaled host-side).
            def relu1(ps, p, sl, in_c2=False):
                lm = _MPERM[p]
                if lm < 2:
                    nc.scalar.activation(
                        h1q[:, lm, sl], ps[:], Relu, bias=bt[:, p : p + 1], scale=16.0
                    )
                elif in_c2 and lm < 6:
                    # tail chunk: write scale-1 e4m3 for the extra fp8 k-pairs
                    # (same op/engine/bias as the bf16 path, fp8 output dtype)
                    nc.vector.tensor_scalar(
                        h1q2[:, lm - 2, :], ps[:], bt[:, p : p + 1], 0.0, add, amax
                    )
                elif p % 2 == 0:
                    nc.vector.tensor_scalar(
                        h1[lm][:, sl], ps[:], bt[:, p : p + 1], 0.0, add, amax
                    )
                else:
                    nc.scalar.activation(
                        h1[lm][:, sl], ps[:], Relu, bias=bt[:, p : p + 1]
                    )

            def relu2(ps, m, sl):
                if m < M2 // 2:
                    nc.vector.tensor_scalar(
                        hf[m][:, sl], ps[:], bt[:, M1 + m : M1 + m + 1], 0.0, add, amax
                    )
                else:
                    nc.scalar.activation(
                        hf[m][:, sl], ps[:], Relu, bias=bt[:, M1 + m : M1 + m + 1]
                    )

            # Phase 1: L1 for ALL chunks (needs only st+w1 ~2.1MB) so w2/w3
            # stream in behind the compute.  Wave order tracks DMA arrival:
            # (m0-3, c0) needs only the "a" halves, (m0-3, c1) adds st_c1,
            # (m4-7, *) needs the "b" halves which land meanwhile.  The
            # small tail chunk (64 cols) runs last, per (m, k).
            big = [c for c in chunks if c[1] == 512]
            small = [c for c in chunks if c[1] < 512]
            for cc0, ccw in big:
                for half in (range(0, 4), range(4, M1)):
                    pss = [pp.tile([128, ccw], f32, name="ps", tag="ps") for _ in half]
                    for k in range(KD):
                        for i, m in enumerate(half):
                            l1_mm(pss[i], m, k, cc0, ccw)
                    for i, m in enumerate(half):
                        relu1(pss[i], m, slice(cc0, cc0 + ccw))
            for m in range(M1):
                for cc0, ccw in small:
                    ps = pp.tile([128, ccw], f32, name="ps", tag="ps")
                    for k in range(KD):
                        l1_mm(ps, m, k, cc0, ccw)
                    relu1(ps, m, slice(cc0, cc0 + ccw), in_c2=True)
            # Phase 2+3: L2 per chunk, then L3 for fully-covered groups with
            # one batched out-DMA per chunk.
            done_j = 0
            for ci, (c0, cw) in enumerate(chunks):
                sl = slice(c0, c0 + cw)
                for m in range(M2):
                    ps = pp.tile([128, cw], f32, name="ps", tag="ps")
                    l2_dr(ps, m, 0, h1q[:, :, sl], True)
                    if cw < 512 and c2w:
                        # alternate DR / bf16 so the 256-wide fp8 LDWEIGHTS
                        # never queue back-to-back (measured ~50ns/pair penalty)
                        l2_mm(ps, m, 6, sl, stop=False)
                        l2_dr(ps, m, 2, h1q2[:, 0:2, :], False)
                        l2_mm(ps, m, 7, sl, stop=False)
                        l2_dr(ps, m, 4, h1q2[:, 2:4, :], False, stop=True)
                        ks = ()
                    else:
                        ks = range(2, K1)
                    for k in ks:
                        l2_mm(ps, m, k, sl)
                    relu2(ps, m, sl)
                lim = c0 + cw
                while done_j < NG and starts[done_j + 1] <= lim:
                    j = done_j
                    sj, ej = int(starts[j]), int(starts[j + 1])
                    # split groups wider than one PSUM bank (512 f32)
                    for g0 in range(sj, ej, 512):
                        g1 = min(g0 + 512, ej)
                        ps = pp3.tile([A, g1 - g0], f32, name="ps3", tag="ps3")
                        for k in range(K2):
                            w0 = P2 + j * K2 * A + k * A
                            nc.tensor.matmul(
                                ps[:],
                                img[:, w0 : w0 + A],
                                hf[k][:, g0:g1],
                                start=(k == 0),
                                stop=(k == K2 - 1),
                            )
                        nc.scalar.activation(
                            osb[:, g0:g1], ps[:], Tanh,
                            bias=bt[:, M1 + M2 + j : M1 + M2 + j + 1],
                        )
                        nc.sync.dma_start(out_ext[:, g0:g1], osb[:, g0:g1])
                    done_j += 1
            assert done_j == NG

    nc.finalize()
    return nc


def _kmajor(w, K):
    """[K*128, F] -> [128, K*F] with col = k*F + f."""
    F = w.shape[1]
    return np.ascontiguousarray(w.reshape(K, 128, F).transpose(1, 0, 2).reshape(128, K * F))


def _prepare(state, idx, W1, b1, W2, b2, W3, b3):
    state = np.ascontiguousarray(np.asarray(state, dtype=np.float32))
    idx = np.asarray(idx)
    W1 = np.asarray(W1, dtype=np.float32)
    b1 = np.asarray(b1, dtype=np.float32)
    W2 = np.asarray(W2, dtype=np.float32)
    b2 = np.asarray(b2, dtype=np.float32)
    W3 = np.asarray(W3, dtype=np.float32)
    b3 = np.asarray(b3, dtype=np.float32)

    B, D = state.shape
    G, _, H1 = W1.shape
    H2 = W2.shape[1]
    A = W3.shape[2]
    KD, K1, K2 = D // 128, H1 // 128, H2 // 128
    M1, M2 = H1 // 128, H2 // 128
    assert G == _NCORES, f"expert-parallel plan assumes {_NCORES} games, got {G}"

    perm, core_groups, M, starts, N = _make_plan(idx, G)
    NG = len(M)
    chunks = _chunks_of(N)

    key = (D, H1, H2, A, NG, tuple(int(x) for x in starts), N)
    if key not in _graph_cache:
        _graph_cache[key] = _build_graph(D, H1, H2, A, NG, starts, N)
    nc = _graph_cache[key]

    # fused image column offsets (must mirror _build_graph)
    c0w0 = chunks[0][1]
    PAIR = c0w0 + H1
    P0 = KD * PAIR
    P1 = P0 + KD * (N - c0w0)
    P2 = P1 + (K1 - 2) * H2
    IMG_W = P2 + NG * K2 * A

    # L2 k0-1 in fp8 (x4096), k2-7 in bf16 (x2^16); b2 carries the 2^16,
    # W3 divides it back out.  All scales are powers of two (exact).
    w2q_f = np.concatenate([W2[:256] * _SW0, W2[256:768] * _SW1])
    w2q = np.clip(w2q_f, -240.0, 240.0).astype(_FP8)
    w2q = np.ascontiguousarray(
        w2q.reshape(6, 128, H2).transpose(1, 0, 2).reshape(128, 6 * H2)
    )
    w2_h = _kmajor((W2[256:] * _SF).astype(_BF16), K1 - 2)
    b2_col = (b2 * _SF).reshape(M2, 128).T.astype(np.float32)

    in_maps = []
    scatters = []  # per core: list of (sorted_positions, col_start)
    for c in range(G):
        sT = np.zeros((D, N), dtype=_BF16)
        img = np.zeros((128, IMG_W), dtype=_BF16)
        bs = np.zeros((128, M1 + M2 + NG), dtype=np.float32)
        bs[:, :M1] = b1[c].reshape(M1, 128)[list(_MPERM)].T
        for p_, lm_ in enumerate(_MPERM):   # relu1 logical m0-1: Relu(16*ps+16*b1)
            if lm_ < 2:
                bs[:, p_] *= _SH
        bs[:, M1 : M1 + M2] = b2_col
        sc = []
        for j, (head, pos) in enumerate(core_groups[c]):
            s0 = int(starts[j])
            if len(pos):
                sT[:, s0 : s0 + len(pos)] = state[perm[pos]].T.astype(_BF16)
                sc.append((pos, s0))
            img[:, P2 + j * K2 * A : P2 + (j + 1) * K2 * A] = (
                (W3[head] / _SF).astype(_BF16)
                .reshape(K2, 128, A).transpose(1, 0, 2).reshape(128, K2 * A)
            )
            bs[:, M1 + M2 + j] = b3[head]
        w1p = W1[c].reshape(D, M1, 128)[:, list(_MPERM), :].reshape(D, H1)
        w1_h = _kmajor(w1p.astype(_BF16), KD)    # [128, KD*H1], col k*H1+m
        for k in range(KD):
            img[:, k * PAIR : k * PAIR + c0w0] = sT[k * 128 : (k + 1) * 128, 0:c0w0]
            img[:, k * PAIR + c0w0 : (k + 1) * PAIR] = w1_h[:, k * H1 : (k + 1) * H1]
        for c0, cw in chunks[1:]:
            img[:, P0 + (c0 - c0w0) * KD : P0 + (c0 - c0w0 + cw) * KD] = (
                sT[:, c0 : c0 + cw].reshape(KD, 128, cw).transpose(1, 0, 2).reshape(128, KD * cw)
            )
        img[:, P1:P2] = w2_h
        in_maps.append({"img": img, "bs": bs, "w2q": w2q})
        scatters.append(sc)
    return nc, in_maps, scatters, (B, A)


def _run(state, idx, W1, b1, W2, b2, W3, b3, trace=False, trace_kwargs=None):
    from concourse.bass_utils import run_bass_kernel_spmd

    nc, in_maps, scatters, (B, A) = _prepare(state, idx, W1, b1, W2, b2, W3, b3)
    res = run_bass_kernel_spmd(
        nc,
        in_maps,
        core_ids=list(range(_NCORES)),
        trace=trace,
        **(trace_kwargs or {}),
    )
    out = np.zeros((B, A), dtype=np.float32)
    for c in range(len(scatters)):
        o = np.asarray(res.results[c]["out"], dtype=np.float32)  # [A, N]
        for pos, s0 in scatters[c]:
            out[pos] = o[:, s0 : s0 + len(pos)].T
    return out, res


def kernel(**inputs) -> np.ndarray:
    out, _ = _run(**inputs)
    return out



# revision 65
# speedup vs baseline: 1.0058x; 1.0004x over previous
"""Trainium2 Bass kernel for nn_ActorNetwork (moe_routing).

Design (host-routed expert parallelism, zero collectives):
  reference semantics: with perm = stable argsort(idx),
    h_f[i] = relu(relu(state[perm[i]] @ W1[g(i)] + b1[g(i)]) @ W2 + b2)
    out[i] = tanh(h_f[i] @ W3[idx[i]] + b3[idx[i]])
  where g(i) (the W1 expert of sorted-position i) depends only on which
  sorted-count block position i falls into.  Core c takes exactly the sorted
  block of game c -> its layer-1 is ONE dense matmul with only W1[c].  Within
  the core, rows are sub-grouped by head game idx[i] so layer-3 is 8 dense
  per-group matmuls.  All routing (gather of state rows in, scatter of output
  rows back) happens on the host during shard/unshard.  Groups are stored
  sorted by size (descending) so the SPMD-uniform slot capacities
  M_j = max_core(j-th largest group) give ~1% padding.

  On-device layout is feature-major: activations live as [feature, row] so
  every matmul is lhsT=weight-tile [K=128, M=128], rhs=activation [K=128,
  N=rows], PSUM out [M features, rows].  Compute dtype bf16, f32 PSUM.
  All inputs are packed on the host into a handful of [128, X] images so each
  one moves with a single large DMA (per-DMA overhead ~0.6us dominates small
  transfers).  State is packed chunk-major so each 512-row chunk is one DMA.

Performance structure (measured at the 2.4GHz PE state):
  - L2's first two k-tiles (K=256 of 1024) run as ONE fp8 e4m3 DoubleRow
    matmul (2x PE rate): h1 m0-1 is written as e4m3 (x16) by scalar ACT,
    w2 k0-1 ships as an fp8 image (x4096), w2 k2-7 stays bf16 pre-scaled
    x2^16 so all PSUM partials share one scale, W3 is pre-scaled x2^-16.
    Costs sqrt(2/8)*3.6% ~= 1.8% quantization error (2e-2 budget), saves
    8N PE column-cycles (~3.7us).
  - PE stream is gapless (<0.5us of stalls): relu work alternates between
    vector (tensor_scalar) and scalar (ACT Relu) by w1-image position
    parity; a 6-deep PSUM pool absorbs wave-boundary handoffs; L1 waves
    run chunk-outer (c0 m0-3, c0 m4-7, c1 ...) tracking DMA arrival.
  - DMA descriptor order is latency-tuned: the first L1 wave's pieces
    (st_c0|w1 "a" halves) lead both HWDGE rings, bias early, w2/w3 last;
    ~74 zero-data warmup matmuls ramp the PE p-state (needs ~3us busy)
    while the first pieces land (~11.3us: boot 7.2 + desc 0.7 + transfer
    + ~2us DMA completion latency).
  - Tail ~4.8us is floor: last tanh ACT + out-DMA desc-gen + ~2us DMA
    completion + NEFF epilogue.  Total ~62.2us vs 47.2us pure-matmul
    floor at bf16 (68.9us before this tuning round).
  NOTE: the shared trn2 pool's PE clock wanders between ~2.4GHz (216ns
  per 512-col matmul) and ~2.0GHz (259ns) run to run; absolute times
  scale by ~1.2x with it.
"""

import numpy as np
import ml_dtypes

_BF16 = ml_dtypes.bfloat16
_FP8 = ml_dtypes.float8_e4m3     # TRN fp8e4: IEEE-ish e4m3, max normal 240
_NCORES = 8
_SH = 16.0                       # h1 fp8 scale (m0-1)
_SW0 = 256.0                     # w2 k0-1 fp8 scale (16*256 = 2^12)
_SW1 = 4096.0                    # w2 k2-5 fp8 scale (h1q2 at scale 1)
_SF = _SH * _SW0                 # 2^12: scale of all L2 partials / hf
# w1 image position -> logical m.  Relu engine alternates by position
# parity (even -> vector, odd -> scalar) so every L1 wave splits its 4
# relus across both engines; logical m0-1 (fp8, scalar ACT) sit at odd
# positions.
_MPERM = (2, 0, 3, 1, 4, 6, 5, 7)
_WARM_MMS = 74  # PE warmup matmuls (~1us cold) to ramp HAM before real work
_graph_cache: dict = {}


def _make_plan(idx: np.ndarray, G: int):
    """Host routing plan: which (sorted-position) rows go to which core/slot."""
    idx = np.asarray(idx)
    perm = np.argsort(idx, kind="stable")
    counts = np.bincount(idx, minlength=G)
    cum = np.zeros(G + 1, dtype=np.int64)
    cum[1:] = np.cumsum(counts)

    core_groups = []  # per core: list of (head_game, sorted_positions) desc by size
    for c in range(G):
        pos = np.arange(cum[c], cum[c + 1])
        heads = idx[pos]
        groups = [(b, pos[heads == b]) for b in range(G)]
        groups.sort(key=lambda t: (-len(t[1]), t[0]))
        core_groups.append(groups)

    sizes = np.array([[len(p) for _, p in groups] for groups in core_groups])
    M = sizes.max(axis=0)          # slot capacity per position (SPMD-uniform)
    keep = M > 0
    M = M[keep]
    core_groups = [[g for g, k in zip(groups, keep) if k] for groups in core_groups]
    NG = len(M)
    starts = np.zeros(NG + 1, dtype=np.int64)
    starts[1:] = np.cumsum(M)
    N = int(starts[-1])
    return perm, core_groups, M, starts, N


def _chunks_of(N):
    # 512-wide chunks: matmuls with 512 moving cols fully hide LDWEIGHTS
    # (~190ns observed at 365 cols vs 213ns at 512 = 25% LDW exposure).
    out = []
    c0 = 0
    while c0 < N:
        cw = min(512, N - c0)
        out.append((c0, cw))
        c0 += cw
    return out


def _build_graph(D, H1, H2, A, NG, starts, N):
    """Build + finalize the SPMD Bass graph (identical for all cores)."""
    from concourse import bacc
    import concourse.mybir as mybir
    from concourse.tile import TileContext

    bf = mybir.dt.bfloat16
    f32 = mybir.dt.float32
    fp8 = mybir.dt.float8e4
    KD, K1, K2 = D // 128, H1 // 128, H2 // 128
    M1, M2 = H1 // 128, H2 // 128
    assert D % 128 == 0 and H1 % 128 == 0 and H2 % 128 == 0 and A == 128

    chunks = _chunks_of(N)
    # One fused bf16 input image per core (single SBUF tile, range-tracked):
    #   [ st_c0_k0 | w1_k0 | ... | st_c0_k3 | w1_k3 | st_c1.. | w2 k2-7 | w3 ]
    # so the critical first-wave pieces stream pairwise on the fast qACT
    # queue with few dma_start instructions (desc-gen ~0.65us each).
    # L2's k0-1 contraction runs in fp8 e4m3 DoubleRow (one matmul covers
    # both k-tiles): h1 m0-1 is stored as fp8 (x16), w2 k0-1 ships as a
    # separate fp8 image (x4096), the bf16 w2 k2-7 is pre-scaled x2^16 so
    # all PSUM partials share one scale, and W3 is pre-scaled x2^-16 to
    # undo it.  Costs ~sqrt(2/8)*3.6% quantization error (total ~1.8%,
    # budget 2%), saves 8N PE column-cycles (~3.7us).
    c0w0 = chunks[0][1]
    PAIR = c0w0 + H1
    P0 = KD * PAIR                      # end of (st_c0 | w1) pairs
    P1 = P0 + KD * (N - c0w0)           # end of st_c1.. ; start of w2 k2-7
    P2 = P1 + (K1 - 2) * H2             # start of w3
    IMG_W = P2 + NG * K2 * A

    def st_col(k, c0, cw):
        if c0 == 0:
            return k * PAIR
        return P0 + (c0 - c0w0) * KD + k * cw

    def w1_col(k):
        return k * PAIR + c0w0

    nc = bacc.Bacc("TRN2")
    img_ext = nc.declare_dram_parameter("img", [128, IMG_W], bf, isOutput=False)
    w2q_ext = nc.declare_dram_parameter("w2q", [128, 6 * H2], fp8, isOutput=False)
    b_ext = nc.declare_dram_parameter("bs", [128, M1 + M2 + NG], f32, isOutput=False)
    out_ext = nc.declare_dram_parameter("out", [A, N], f32, isOutput=True)

    add = mybir.AluOpType.add
    amax = mybir.AluOpType.max
    Tanh = mybir.ActivationFunctionType.Tanh
    Relu = mybir.ActivationFunctionType.Relu

    with TileContext(nc) as tc:
        with (
            tc.tile_pool(name="weights", bufs=1) as wp,
            tc.tile_pool(name="acts", bufs=1) as ap,
            tc.tile_pool(name="psum", bufs=6, space="PSUM") as pp,
            tc.tile_pool(name="psum3", bufs=2, space="PSUM") as pp3,
        ):
            img = wp.tile([128, IMG_W], bf, name="img", tag="img")
            w2qt = wp.tile([128, 6, H2], fp8, name="w2qt", tag="w2qt")
            bt = wp.tile([128, M1 + M2 + NG], f32, name="bt", tag="bt")
            h1q = ap.tile([128, 2, N], fp8, name="h1q", tag="h1q")
            c2w = chunks[-1][1] if chunks[-1][1] < 512 else 0
            # the small tail chunk runs L2 k0-5 in fp8 (3 DoubleRow matmuls):
            # its h1 m2-5 lives here as e4m3 at scale 1 (vector relu writes it
            # with the plain bias; w2 k2-5 carries the full 2^12)
            h1q2 = ap.tile([128, 4, c2w], fp8, name="h1q2", tag="h1q2") if c2w else None
            h1 = [ap.tile([128, N], bf, name=f"h1_{m}", tag=f"h1_{m}") if m >= 2 else None
                  for m in range(M1)]
            hf = [ap.tile([128, N], bf, name=f"hf_{m}", tag=f"hf_{m}") for m in range(M2)]
            osb = ap.tile([A, N], f32, name="osb", tag="osb")
            wrm = wp.tile([128, 64], bf, name="wrm", tag="wrm")

            # PE warmup: ramp the HAM clock gate to 8/8 while DMAs land.
            nc.gpsimd.memset(wrm[:], 0)
            pw = pp3.tile([64, 64], f32, name="psw", tag="ps3")
            for _ in range(_WARM_MMS):
                nc.tensor.matmul(pw[:], wrm[:, :64], wrm[:], start=True, stop=True)

            # DMAs: descriptor-gen is ~0.6us per dma_start per queue and the
            # two HWDGE rings share the ~436GB/s AXI ports, so ORDER is what
            # matters: land the first L1 wave's data (st_c0|w1 m0-3 "a"
            # halves of every k) before any "b" half, bias early, then state
            # c1.., then w2 (scalar ring) / w3 (sync ring) behind everything.
            def amid(k):
                return k * PAIR + c0w0 + H1 // 2

            for k in range(KD):                                             # a halves
                eng = nc.sync if k % 2 == 0 else nc.scalar
                eng.dma_start(img[:, k * PAIR : amid(k)], img_ext[:, k * PAIR : amid(k)])
            nc.sync.dma_start(bt[:], b_ext[:])
            stm = P0 + (P1 - P0) // 2
            if P1 > P0:
                nc.scalar.dma_start(img[:, P0:stm], img_ext[:, P0:stm])    # st_c1 k0-1
            # b halves; scalar's desc burst must end early (it also runs the
            # odd-position relu1 ACTs from ~13.5us), so sync carries most of
            # the mid-priority pieces
            nc.sync.dma_start(img[:, amid(0) : PAIR], img_ext[:, amid(0) : PAIR])
            nc.scalar.dma_start(img[:, amid(1) : 2 * PAIR], img_ext[:, amid(1) : 2 * PAIR])
            nc.sync.dma_start(img[:, amid(2) : 3 * PAIR], img_ext[:, amid(2) : 3 * PAIR])
            if P1 > stm:
                nc.sync.dma_start(img[:, stm:P1], img_ext[:, stm:P1])      # st_c1 k2-3, c2
            nc.scalar.dma_start(img[:, amid(3) : 4 * PAIR], img_ext[:, amid(3) : 4 * PAIR])
            nc.sync.dma_start(w2qt[:, :, :], w2q_ext[:])                   # w2 k0-1 fp8
            half2 = P1 + ((K1 - 2) // 2) * H2
            nc.scalar.dma_start(img[:, P1:half2], img_ext[:, P1:half2])    # w2 k2-4
            nc.scalar.dma_start(img[:, half2:P2], img_ext[:, half2:P2])    # w2 k5-7
            nc.sync.dma_start(img[:, P2:IMG_W], img_ext[:, P2:IMG_W])      # w3

            def l1_mm(ps, m, k, c0, cw):
                w0 = w1_col(k) + m * 128
                s0 = st_col(k, c0, cw)
                nc.tensor.matmul(
                    ps[:],
                    img[:, w0 : w0 + 128],
                    img[:, s0 : s0 + cw],
                    start=(k == 0),
                    stop=(k == KD - 1),
                )

            DR = mybir.MatmulPerfMode.DoubleRow

            def l2_dr(ps, m, kk, rhs, start):
                # fp8 DoubleRow: one matmul contracts k-tiles kk and kk+1
                nc.tensor.matmul(
                    ps[:],
                    w2qt[:, kk : kk + 2, m * 128 : (m + 1) * 128],
                    rhs,
                    start=start,
                    stop=False,
                    perf_mode=DR,
                    skip_group_check=True,
                )

            def l2_mm(ps, m, k, sl):
                w0 = P1 + (k - 2) * H2 + m * 128
                nc.tensor.matmul(
                    ps[:],
                    img[:, w0 : w0 + 128],
                    h1[k][:, sl],
                    start=False,
                    stop=(k == K1 - 1),
                    skip_group_check=True,
                )

            # relu split: image positions 0-3 (first wave) on vector,
            # positions 4-7 (second wave) on scalar ACT — halves the
            # elementwise backlog that would block PSUM-pool recycling.
            # Logical m0-1 are fp8: out = Relu(16*ps + 16*b1) cast to e4m3
            # (bias pre-scaled host-side).
            def relu1(ps, p, sl, in_c2=False):
                lm = _MPERM[p]
                if lm < 2:
                    nc.scalar.activation(
                        h1q[:, lm, sl], ps[:], Relu, bias=bt[:, p : p + 1], scale=16.0
                    )
                elif in_c2 and lm < 6:
                    # tail chunk: write scale-1 e4m3 for the extra fp8 k-pairs
                    # (same op/engine/bias as the bf16 path, fp8 output dtype)
                    nc.vector.tensor_scalar(
                        h1q2[:, lm - 2, :], ps[:], bt[:, p : p + 1], 0.0, add, amax
                    )
                elif p % 2 == 0:
                    nc.vector.tensor_scalar(
                        h1[lm][:, sl], ps[:], bt[:, p : p + 1], 0.0, add, amax
                    )
                else:
                    nc.scalar.activation(
                        h1[lm][:, sl], ps[:], Relu, bias=bt[:, p : p + 1]
                    )

            def relu2(ps, m, sl):
                if m < M2 // 2:
                    nc.vector.tensor_scalar(
                        hf[m][:, sl], ps[:], bt[:, M1 + m : M1 + m + 1], 0.0, add, amax
                    )
                else:
                    nc.scalar.activation(
                        hf[m][:, sl], ps[:], Relu, bias=bt[:, M1 + m : M1 + m + 1]
                    )

            # Phase 1: L1 for ALL chunks (needs only st+w1 ~2.1MB) so w2/w3
            # stream in behind the compute.  Wave order tracks DMA arrival:
            # (m0-3, c0) needs only the "a" halves, (m0-3, c1) adds st_c1,
            # (m4-7, *) needs the "b" halves which land meanwhile.  The
            # small tail chunk (64 cols) runs last, per (m, k).
            big = [c for c in chunks if c[1] == 512]
            small = [c for c in chunks if c[1] < 512]
            for cc0, ccw in big:
                for half in (range(0, 4), range(4, M1)):
                    pss = [pp.tile([128, ccw], f32, name="ps", tag="ps") for _ in half]
                    for k in range(KD):
                        for i, m in enumerate(half):
                            l1_mm(pss[i], m, k, cc0, ccw)
                    for i, m in enumerate(half):
                        relu1(pss[i], m, slice(cc0, cc0 + ccw))
            for m in range(M1):
                for cc0, ccw in small:
                    ps = pp.tile([128, ccw], f32, name="ps", tag="ps")
                    for k in range(KD):
                        l1_mm(ps, m, k, cc0, ccw)
                    relu1(ps, m, slice(cc0, cc0 + ccw), in_c2=True)
            # Phase 2+3: L2 per chunk, then L3 for fully-covered groups with
            # one batched out-DMA per chunk.
            done_j = 0
            for ci, (c0, cw) in enumerate(chunks):
                sl = slice(c0, c0 + cw)
                for m in range(M2):
                    ps = pp.tile([128, cw], f32, name="ps", tag="ps")
                    l2_dr(ps, m, 0, h1q[:, :, sl], True)
                    if cw < 512 and c2w:
                        l2_dr(ps, m, 2, h1q2[:, 0:2, :], False)
                        l2_dr(ps, m, 4, h1q2[:, 2:4, :], False)
                        ks = range(6, K1)
                    else:
                        ks = range(2, K1)
                    for k in ks:
                        l2_mm(ps, m, k, sl)
                    relu2(ps, m, sl)
                lim = c0 + cw
                while done_j < NG and starts[done_j + 1] <= lim:
                    j = done_j
                    sj, ej = int(starts[j]), int(starts[j + 1])
                    # split groups wider than one PSUM bank (512 f32)
                    for g0 in range(sj, ej, 512):
                        g1 = min(g0 + 512, ej)
                        ps = pp3.tile([A, g1 - g0], f32, name="ps3", tag="ps3")
                        for k in range(K2):
                            w0 = P2 + j * K2 * A + k * A
                            nc.tensor.matmul(
                                ps[:],
                                img[:, w0 : w0 + A],
                                hf[k][:, g0:g1],
                                start=(k == 0),
                                stop=(k == K2 - 1),
                            )
                        nc.scalar.activation(
                            osb[:, g0:g1], ps[:], Tanh,
                            bias=bt[:, M1 + M2 + j : M1 + M2 + j + 1],
                        )
                        nc.sync.dma_start(out_ext[:, g0:g1], osb[:, g0:g1])
                    done_j += 1
            assert done_j == NG

    nc.finalize()
    return nc


def _kmajor(w, K):
    """[K*128, F] -> [128, K*F] with col = k*F + f."""
    F = w.shape[1]
    return np.ascontiguousarray(w.reshape(K, 128, F).transpose(1, 0, 2).reshape(128, K * F))


def _prepare(state, idx, W1, b1, W2, b2, W3, b3):
    state = np.ascontiguousarray(np.asarray(state, dtype=np.float32))
    idx = np.asarray(idx)
    W1 = np.asarray(W1, dtype=np.float32)
    b1 = np.asarray(b1, dtype=np.float32)
    W2 = np.asarray(W2, dtype=np.float32)
    b2 = np.asarray(b2, dtype=np.float32)
    W3 = np.asarray(W3, dtype=np.float32)
    b3 = np.asarray(b3, dtype=np.float32)

    B, D = state.shape
    G, _, H1 = W1.shape
    H2 = W2.shape[1]
    A = W3.shape[2]
    KD, K1, K2 = D // 128, H1 // 128, H2 // 128
    M1, M2 = H1 // 128, H2 // 128
    assert G == _NCORES, f"expert-parallel plan assumes {_NCORES} games, got {G}"

    perm, core_groups, M, starts, N = _make_plan(idx, G)
    NG = len(M)
    chunks = _chunks_of(N)

    key = (D, H1, H2, A, NG, tuple(int(x) for x in starts), N)
    if key not in _graph_cache:
        _graph_cache[key] = _build_graph(D, H1, H2, A, NG, starts, N)
    nc = _graph_cache[key]

    # fused image column offsets (must mirror _build_graph)
    c0w0 = chunks[0][1]
    PAIR = c0w0 + H1
    P0 = KD * PAIR
    P1 = P0 + KD * (N - c0w0)
    P2 = P1 + (K1 - 2) * H2
    IMG_W = P2 + NG * K2 * A

    # L2 k0-1 in fp8 (x4096), k2-7 in bf16 (x2^16); b2 carries the 2^16,
    # W3 divides it back out.  All scales are powers of two (exact).
    w2q_f = np.concatenate([W2[:256] * _SW0, W2[256:768] * _SW1])
    w2q = np.clip(w2q_f, -240.0, 240.0).astype(_FP8)
    w2q = np.ascontiguousarray(
        w2q.reshape(6, 128, H2).transpose(1, 0, 2).reshape(128, 6 * H2)
    )
    w2_h = _kmajor((W2[256:] * _SF).astype(_BF16), K1 - 2)
    b2_col = (b2 * _SF).reshape(M2, 128).T.astype(np.float32)

    in_maps = []
    scatters = []  # per core: list of (sorted_positions, col_start)
    for c in range(G):
        sT = np.zeros((D, N), dtype=_BF16)
        img = np.zeros((128, IMG_W), dtype=_BF16)
        bs = np.zeros((128, M1 + M2 + NG), dtype=np.float32)
        bs[:, :M1] = b1[c].reshape(M1, 128)[list(_MPERM)].T
        for p_, lm_ in enumerate(_MPERM):   # relu1 logical m0-1: Relu(16*ps+16*b1)
            if lm_ < 2:
                bs[:, p_] *= _SH
        bs[:, M1 : M1 + M2] = b2_col
        sc = []
        for j, (head, pos) in enumerate(core_groups[c]):
            s0 = int(starts[j])
            if len(pos):
                sT[:, s0 : s0 + len(pos)] = state[perm[pos]].T.astype(_BF16)
                sc.append((pos, s0))
            img[:, P2 + j * K2 * A : P2 + (j + 1) * K2 * A] = (
                (W3[head] / _SF).astype(_BF16)
                .reshape(K2, 128, A).transpose(1, 0, 2).reshape(128, K2 * A)
            )
            bs[:, M1 + M2 + j] = b3[head]
        w1p = W1[c].reshape(D, M1, 128)[:, list(_MPERM), :].reshape(D, H1)
        w1_h = _kmajor(w1p.astype(_BF16), KD)    # [128, KD*H1], col k*H1+m
        for k in range(KD):
            img[:, k * PAIR : k * PAIR + c0w0] = sT[k * 128 : (k + 1) * 128, 0:c0w0]
            img[:, k * PAIR + c0w0 : (k + 1) * PAIR] = w1_h[:, k * H1 : (k + 1) * H1]
        for c0, cw in chunks[1:]:
            img[:, P0 + (c0 - c0w0) * KD : P0 + (c0 - c0w0 + cw) * KD] = (
                sT[:, c0 : c0 + cw].reshape(KD, 128, cw).transpose(1, 0, 2).reshape(128, KD * cw)
            )
        img[:, P1:P2] = w2_h
        in_maps.append({"img": img, "bs": bs, "w2q": w2q})
        scatters.append(sc)
    return nc, in_maps, scatters, (B, A)


def _run(state, idx, W1, b1, W2, b2, W3, b3, trace=False, trace_kwargs=None):
    from concourse.bass_utils import run_bass_kernel_spmd

    nc, in_maps, scatters, (B, A) = _prepare(state, idx, W1, b1, W2, b2, W3, b3)
    res = run_bass_kernel_spmd(
        nc,
        in_maps,
        core_ids=list(range(_NCORES)),
        trace=trace,
        **(trace_kwargs or {}),
    )
    out = np.zeros((B, A), dtype=np.float32)
    for c in range(len(scatters)):
        o = np.asarray(res.results[c]["out"], dtype=np.float32)  # [A, N]
        for pos, s0 in scatters[c]:
            out[pos] = o[:, s0 : s0 + len(pos)].T
    return out, res


def kernel(**inputs) -> np.ndarray:
    out, _ = _run(**inputs)
    return out



# revision 66
# speedup vs baseline: 1.0123x; 1.0064x over previous
"""Trainium2 Bass kernel for nn_ActorNetwork (moe_routing).

Design (host-routed expert parallelism, zero collectives):
  reference semantics: with perm = stable argsort(idx),
    h_f[i] = relu(relu(state[perm[i]] @ W1[g(i)] + b1[g(i)]) @ W2 + b2)
    out[i] = tanh(h_f[i] @ W3[idx[i]] + b3[idx[i]])
  where g(i) (the W1 expert of sorted-position i) depends only on which
  sorted-count block position i falls into.  Core c takes exactly the sorted
  block of game c -> its layer-1 is ONE dense matmul with only W1[c].  Within
  the core, rows are sub-grouped by head game idx[i] so layer-3 is 8 dense
  per-group matmuls.  All routing (gather of state rows in, scatter of output
  rows back) happens on the host during shard/unshard.  Groups are stored
  sorted by size (descending) so the SPMD-uniform slot capacities
  M_j = max_core(j-th largest group) give ~1% padding.

  On-device layout is feature-major: activations live as [feature, row] so
  every matmul is lhsT=weight-tile [K=128, M=128], rhs=activation [K=128,
  N=rows], PSUM out [M features, rows].  Compute dtype bf16, f32 PSUM.
  All inputs are packed on the host into a handful of [128, X] images so each
  one moves with a single large DMA (per-DMA overhead ~0.6us dominates small
  transfers).  State is packed chunk-major so each 512-row chunk is one DMA.

Performance structure (measured at the 2.4GHz PE state):
  - L2's first two k-tiles (K=256 of 1024) run as ONE fp8 e4m3 DoubleRow
    matmul (2x PE rate): h1 m0-1 is written as e4m3 (x16) by scalar ACT,
    w2 k0-1 ships as an fp8 image (x4096), w2 k2-7 stays bf16 pre-scaled
    x2^16 so all PSUM partials share one scale, W3 is pre-scaled x2^-16.
    Costs sqrt(2/8)*3.6% ~= 1.8% quantization error (2e-2 budget), saves
    8N PE column-cycles (~3.7us).
  - PE stream is gapless (<0.5us of stalls): relu work alternates between
    vector (tensor_scalar) and scalar (ACT Relu) by w1-image position
    parity; a 6-deep PSUM pool absorbs wave-boundary handoffs; L1 waves
    run chunk-outer (c0 m0-3, c0 m4-7, c1 ...) tracking DMA arrival.
  - DMA descriptor order is latency-tuned: the first L1 wave's pieces
    (st_c0|w1 "a" halves) lead both HWDGE rings, bias early, w2/w3 last;
    ~74 zero-data warmup matmuls ramp the PE p-state (needs ~3us busy)
    while the first pieces land (~11.3us: boot 7.2 + desc 0.7 + transfer
    + ~2us DMA completion latency).
  - Tail ~4.8us is floor: last tanh ACT + out-DMA desc-gen + ~2us DMA
    completion + NEFF epilogue.  Total ~62.2us vs 47.2us pure-matmul
    floor at bf16 (68.9us before this tuning round).
  NOTE: the shared trn2 pool's PE clock wanders between ~2.4GHz (216ns
  per 512-col matmul) and ~2.0GHz (259ns) run to run; absolute times
  scale by ~1.2x with it.
"""

import numpy as np
import ml_dtypes

_BF16 = ml_dtypes.bfloat16
_FP8 = ml_dtypes.float8_e4m3     # TRN fp8e4: IEEE-ish e4m3, max normal 240
_NCORES = 8
_SH = 16.0                       # h1 fp8 scale (m0-1)
_SW0 = 256.0                     # w2 k0-1 fp8 scale (16*256 = 2^12)
_SW1 = 4096.0                    # w2 k2-5 fp8 scale (h1q2 at scale 1)
_SF = _SH * _SW0                 # 2^12: scale of all L2 partials / hf
# w1 image position -> logical m.  Relu engine alternates by position
# parity (even -> vector, odd -> scalar) so every L1 wave splits its 4
# relus across both engines; logical m0-1 (fp8, scalar ACT) sit at odd
# positions.
_MPERM = (2, 0, 3, 1, 4, 6, 5, 7)
_WARM_MMS = 74  # PE warmup matmuls (~1us cold) to ramp HAM before real work
_graph_cache: dict = {}


def _make_plan(idx: np.ndarray, G: int):
    """Host routing plan: which (sorted-position) rows go to which core/slot."""
    idx = np.asarray(idx)
    perm = np.argsort(idx, kind="stable")
    counts = np.bincount(idx, minlength=G)
    cum = np.zeros(G + 1, dtype=np.int64)
    cum[1:] = np.cumsum(counts)

    core_groups = []  # per core: list of (head_game, sorted_positions) desc by size
    for c in range(G):
        pos = np.arange(cum[c], cum[c + 1])
        heads = idx[pos]
        groups = [(b, pos[heads == b]) for b in range(G)]
        groups.sort(key=lambda t: (-len(t[1]), t[0]))
        core_groups.append(groups)

    sizes = np.array([[len(p) for _, p in groups] for groups in core_groups])
    M = sizes.max(axis=0)          # slot capacity per position (SPMD-uniform)
    keep = M > 0
    M = M[keep]
    core_groups = [[g for g, k in zip(groups, keep) if k] for groups in core_groups]
    NG = len(M)
    starts = np.zeros(NG + 1, dtype=np.int64)
    starts[1:] = np.cumsum(M)
    N = int(starts[-1])
    return perm, core_groups, M, starts, N


def _chunks_of(N):
    # 512-wide chunks: matmuls with 512 moving cols fully hide LDWEIGHTS
    # (~190ns observed at 365 cols vs 213ns at 512 = 25% LDW exposure).
    out = []
    c0 = 0
    while c0 < N:
        cw = min(512, N - c0)
        out.append((c0, cw))
        c0 += cw
    return out


def _build_graph(D, H1, H2, A, NG, starts, N):
    """Build + finalize the SPMD Bass graph (identical for all cores)."""
    from concourse import bacc
    import concourse.mybir as mybir
    from concourse.tile import TileContext

    bf = mybir.dt.bfloat16
    f32 = mybir.dt.float32
    fp8 = mybir.dt.float8e4
    KD, K1, K2 = D // 128, H1 // 128, H2 // 128
    M1, M2 = H1 // 128, H2 // 128
    assert D % 128 == 0 and H1 % 128 == 0 and H2 % 128 == 0 and A == 128

    chunks = _chunks_of(N)
    # One fused bf16 input image per core (single SBUF tile, range-tracked):
    #   [ st_c0_k0 | w1_k0 | ... | st_c0_k3 | w1_k3 | st_c1.. | w2 k2-7 | w3 ]
    # so the critical first-wave pieces stream pairwise on the fast qACT
    # queue with few dma_start instructions (desc-gen ~0.65us each).
    # L2's k0-1 contraction runs in fp8 e4m3 DoubleRow (one matmul covers
    # both k-tiles): h1 m0-1 is stored as fp8 (x16), w2 k0-1 ships as a
    # separate fp8 image (x4096), the bf16 w2 k2-7 is pre-scaled x2^16 so
    # all PSUM partials share one scale, and W3 is pre-scaled x2^-16 to
    # undo it.  Costs ~sqrt(2/8)*3.6% quantization error (total ~1.8%,
    # budget 2%), saves 8N PE column-cycles (~3.7us).
    c0w0 = chunks[0][1]
    PAIR = c0w0 + H1
    P0 = KD * PAIR                      # end of (st_c0 | w1) pairs
    P1 = P0 + KD * (N - c0w0)           # end of st_c1.. ; start of w2 k2-7
    P2 = P1 + (K1 - 2) * H2             # start of w3
    IMG_W = P2 + NG * K2 * A

    def st_col(k, c0, cw):
        if c0 == 0:
            return k * PAIR
        return P0 + (c0 - c0w0) * KD + k * cw

    def w1_col(k):
        return k * PAIR + c0w0

    nc = bacc.Bacc("TRN2")
    img_ext = nc.declare_dram_parameter("img", [128, IMG_W], bf, isOutput=False)
    w2q_ext = nc.declare_dram_parameter("w2q", [128, 6 * H2], fp8, isOutput=False)
    b_ext = nc.declare_dram_parameter("bs", [128, M1 + M2 + NG], f32, isOutput=False)
    out_ext = nc.declare_dram_parameter("out", [A, N], f32, isOutput=True)

    add = mybir.AluOpType.add
    amax = mybir.AluOpType.max
    Tanh = mybir.ActivationFunctionType.Tanh
    Relu = mybir.ActivationFunctionType.Relu

    with TileContext(nc) as tc:
        with (
            tc.tile_pool(name="weights", bufs=1) as wp,
            tc.tile_pool(name="acts", bufs=1) as ap,
            tc.tile_pool(name="psum", bufs=6, space="PSUM") as pp,
            tc.tile_pool(name="psum3", bufs=2, space="PSUM") as pp3,
        ):
            img = wp.tile([128, IMG_W], bf, name="img", tag="img")
            w2qt = wp.tile([128, 6, H2], fp8, name="w2qt", tag="w2qt")
            bt = wp.tile([128, M1 + M2 + NG], f32, name="bt", tag="bt")
            h1q = ap.tile([128, 2, N], fp8, name="h1q", tag="h1q")
            c2w = chunks[-1][1] if chunks[-1][1] < 512 else 0
            # the small tail chunk runs L2 k0-5 in fp8 (3 DoubleRow matmuls):
            # its h1 m2-5 lives here as e4m3 at scale 1 (vector relu writes it
            # with the plain bias; w2 k2-5 carries the full 2^12)
            h1q2 = ap.tile([128, 4, c2w], fp8, name="h1q2", tag="h1q2") if c2w else None
            h1 = [ap.tile([128, N], bf, name=f"h1_{m}", tag=f"h1_{m}") if m >= 2 else None
                  for m in range(M1)]
            hf = [ap.tile([128, N], bf, name=f"hf_{m}", tag=f"hf_{m}") for m in range(M2)]
            osb = ap.tile([A, N], f32, name="osb", tag="osb")
            wrm = wp.tile([128, 64], bf, name="wrm", tag="wrm")

            # PE warmup: ramp the HAM clock gate to 8/8 while DMAs land.
            nc.gpsimd.memset(wrm[:], 0)
            pw = pp3.tile([64, 64], f32, name="psw", tag="ps3")
            for _ in range(_WARM_MMS):
                nc.tensor.matmul(pw[:], wrm[:, :64], wrm[:], start=True, stop=True)

            # DMAs: descriptor-gen is ~0.6us per dma_start per queue and the
            # two HWDGE rings share the ~436GB/s AXI ports, so ORDER is what
            # matters: land the first L1 wave's data (st_c0|w1 m0-3 "a"
            # halves of every k) before any "b" half, bias early, then state
            # c1.., then w2 (scalar ring) / w3 (sync ring) behind everything.
            def amid(k):
                return k * PAIR + c0w0 + H1 // 2

            for k in range(KD):                                             # a halves
                eng = nc.sync if k % 2 == 0 else nc.scalar
                eng.dma_start(img[:, k * PAIR : amid(k)], img_ext[:, k * PAIR : amid(k)])
            nc.sync.dma_start(bt[:], b_ext[:])
            stm = P0 + (P1 - P0) // 2
            if P1 > P0:
                nc.scalar.dma_start(img[:, P0:stm], img_ext[:, P0:stm])    # st_c1 k0-1
            # b halves; scalar's desc burst must end early (it also runs the
            # odd-position relu1 ACTs from ~13.5us), so sync carries most of
            # the mid-priority pieces
            nc.sync.dma_start(img[:, amid(0) : PAIR], img_ext[:, amid(0) : PAIR])
            nc.scalar.dma_start(img[:, amid(1) : 2 * PAIR], img_ext[:, amid(1) : 2 * PAIR])
            nc.sync.dma_start(img[:, amid(2) : 3 * PAIR], img_ext[:, amid(2) : 3 * PAIR])
            if P1 > stm:
                nc.sync.dma_start(img[:, stm:P1], img_ext[:, stm:P1])      # st_c1 k2-3, c2
            nc.scalar.dma_start(img[:, amid(3) : 4 * PAIR], img_ext[:, amid(3) : 4 * PAIR])
            nc.sync.dma_start(w2qt[:, :, :], w2q_ext[:])                   # w2 k0-1 fp8
            half2 = P1 + ((K1 - 2) // 2) * H2
            nc.scalar.dma_start(img[:, P1:half2], img_ext[:, P1:half2])    # w2 k2-4
            nc.scalar.dma_start(img[:, half2:P2], img_ext[:, half2:P2])    # w2 k5-7
            nc.sync.dma_start(img[:, P2:IMG_W], img_ext[:, P2:IMG_W])      # w3

            def l1_mm(ps, m, k, c0, cw):
                w0 = w1_col(k) + m * 128
                s0 = st_col(k, c0, cw)
                nc.tensor.matmul(
                    ps[:],
                    img[:, w0 : w0 + 128],
                    img[:, s0 : s0 + cw],
                    start=(k == 0),
                    stop=(k == KD - 1),
                )

            DR = mybir.MatmulPerfMode.DoubleRow

            def l2_dr(ps, m, kk, rhs, start):
                # fp8 DoubleRow: one matmul contracts k-tiles kk and kk+1
                nc.tensor.matmul(
                    ps[:],
                    w2qt[:, kk : kk + 2, m * 128 : (m + 1) * 128],
                    rhs,
                    start=start,
                    stop=False,
                    perf_mode=DR,
                    skip_group_check=True,
                )

            def l2_mm(ps, m, k, sl):
                w0 = P1 + (k - 2) * H2 + m * 128
                nc.tensor.matmul(
                    ps[:],
                    img[:, w0 : w0 + 128],
                    h1[k][:, sl],
                    start=False,
                    stop=(k == K1 - 1),
                    skip_group_check=True,
                )

            # relu split: image positions 0-3 (first wave) on vector,
            # positions 4-7 (second wave) on scalar ACT — halves the
            # elementwise backlog that would block PSUM-pool recycling.
            # Logical m0-1 are fp8: out = Relu(16*ps + 16*b1) cast to e4m3
            # (bias pre-scaled host-side).
            def relu1(ps, p, sl, in_c2=False):
                lm = _MPERM[p]
                if lm < 2:
                    nc.scalar.activation(
                        h1q[:, lm, sl], ps[:], Relu, bias=bt[:, p : p + 1], scale=16.0
                    )
                elif in_c2 and lm < 6:
                    # tail chunk: write scale-1 e4m3 for the extra fp8 k-pairs
                    # (same op/engine/bias as the bf16 path, fp8 output dtype)
                    nc.vector.tensor_scalar(
                        h1q2[:, lm - 2, :], ps[:], bt[:, p : p + 1], 0.0, add, amax
                    )
                elif p % 2 == 0:
                    nc.vector.tensor_scalar(
                        h1[lm][:, sl], ps[:], bt[:, p : p + 1], 0.0, add, amax
                    )
                else:
                    nc.scalar.activation(
                        h1[lm][:, sl], ps[:], Relu, bias=bt[:, p : p + 1]
                    )

            def relu2(ps, m, sl):
                # tail chunk: vector also takes m4-5 (scalar is busy with the
                # preceding chunk's L3 tanh ACTs there; 953ns stall otherwise)
                if m < M2 // 2 or (m < 6 and (sl.stop - sl.start) < 512):
                    nc.vector.tensor_scalar(
                        hf[m][:, sl], ps[:], bt[:, M1 + m : M1 + m + 1], 0.0, add, amax
                    )
                else:
                    nc.scalar.activation(
                        hf[m][:, sl], ps[:], Relu, bias=bt[:, M1 + m : M1 + m + 1]
                    )

            # Phase 1: L1 for ALL chunks (needs only st+w1 ~2.1MB) so w2/w3
            # stream in behind the compute.  Wave order tracks DMA arrival:
            # (m0-3, c0) needs only the "a" halves, (m0-3, c1) adds st_c1,
            # (m4-7, *) needs the "b" halves which land meanwhile.  The
            # small tail chunk (64 cols) runs last, per (m, k).
            big = [c for c in chunks if c[1] == 512]
            small = [c for c in chunks if c[1] < 512]
            for cc0, ccw in big:
                for half in (range(0, 4), range(4, M1)):
                    pss = [pp.tile([128, ccw], f32, name="ps", tag="ps") for _ in half]
                    for k in range(KD):
                        for i, m in enumerate(half):
                            l1_mm(pss[i], m, k, cc0, ccw)
                    for i, m in enumerate(half):
                        relu1(pss[i], m, slice(cc0, cc0 + ccw))
            for m in range(M1):
                for cc0, ccw in small:
                    ps = pp.tile([128, ccw], f32, name="ps", tag="ps")
                    for k in range(KD):
                        l1_mm(ps, m, k, cc0, ccw)
                    relu1(ps, m, slice(cc0, cc0 + ccw), in_c2=True)
            # Phase 2+3: L2 per chunk, then L3 for fully-covered groups with
            # one batched out-DMA per chunk.
            done_j = 0
            for ci, (c0, cw) in enumerate(chunks):
                sl = slice(c0, c0 + cw)
                for m in range(M2):
                    ps = pp.tile([128, cw], f32, name="ps", tag="ps")
                    l2_dr(ps, m, 0, h1q[:, :, sl], True)
                    if cw < 512 and c2w:
                        l2_dr(ps, m, 2, h1q2[:, 0:2, :], False)
                        l2_dr(ps, m, 4, h1q2[:, 2:4, :], False)
                        ks = range(6, K1)
                    else:
                        ks = range(2, K1)
                    for k in ks:
                        l2_mm(ps, m, k, sl)
                    relu2(ps, m, sl)
                lim = c0 + cw
                while done_j < NG and starts[done_j + 1] <= lim:
                    j = done_j
                    sj, ej = int(starts[j]), int(starts[j + 1])
                    # split groups wider than one PSUM bank (512 f32)
                    for g0 in range(sj, ej, 512):
                        g1 = min(g0 + 512, ej)
                        ps = pp3.tile([A, g1 - g0], f32, name="ps3", tag="ps3")
                        for k in range(K2):
                            w0 = P2 + j * K2 * A + k * A
                            nc.tensor.matmul(
                                ps[:],
                                img[:, w0 : w0 + A],
                                hf[k][:, g0:g1],
                                start=(k == 0),
                                stop=(k == K2 - 1),
                            )
                        nc.scalar.activation(
                            osb[:, g0:g1], ps[:], Tanh,
                            bias=bt[:, M1 + M2 + j : M1 + M2 + j + 1],
                        )
                        nc.sync.dma_start(out_ext[:, g0:g1], osb[:, g0:g1])
                    done_j += 1
            assert done_j == NG

    nc.finalize()
    return nc


def _kmajor(w, K):
    """[K*128, F] -> [128, K*F] with col = k*F + f."""
    F = w.shape[1]
    return np.ascontiguousarray(w.reshape(K, 128, F).transpose(1, 0, 2).reshape(128, K * F))


def _prepare(state, idx, W1, b1, W2, b2, W3, b3):
    state = np.ascontiguousarray(np.asarray(state, dtype=np.float32))
    idx = np.asarray(idx)
    W1 = np.asarray(W1, dtype=np.float32)
    b1 = np.asarray(b1, dtype=np.float32)
    W2 = np.asarray(W2, dtype=np.float32)
    b2 = np.asarray(b2, dtype=np.float32)
    W3 = np.asarray(W3, dtype=np.float32)
    b3 = np.asarray(b3, dtype=np.float32)

    B, D = state.shape
    G, _, H1 = W1.shape
    H2 = W2.shape[1]
    A = W3.shape[2]
    KD, K1, K2 = D // 128, H1 // 128, H2 // 128
    M1, M2 = H1 // 128, H2 // 128
    assert G == _NCORES, f"expert-parallel plan assumes {_NCORES} games, got {G}"

    perm, core_groups, M, starts, N = _make_plan(idx, G)
    NG = len(M)
    chunks = _chunks_of(N)

    key = (D, H1, H2, A, NG, tuple(int(x) for x in starts), N)
    if key not in _graph_cache:
        _graph_cache[key] = _build_graph(D, H1, H2, A, NG, starts, N)
    nc = _graph_cache[key]

    # fused image column offsets (must mirror _build_graph)
    c0w0 = chunks[0][1]
    PAIR = c0w0 + H1
    P0 = KD * PAIR
    P1 = P0 + KD * (N - c0w0)
    P2 = P1 + (K1 - 2) * H2
    IMG_W = P2 + NG * K2 * A

    # L2 k0-1 in fp8 (x4096), k2-7 in bf16 (x2^16); b2 carries the 2^16,
    # W3 divides it back out.  All scales are powers of two (exact).
    w2q_f = np.concatenate([W2[:256] * _SW0, W2[256:768] * _SW1])
    w2q = np.clip(w2q_f, -240.0, 240.0).astype(_FP8)
    w2q = np.ascontiguousarray(
        w2q.reshape(6, 128, H2).transpose(1, 0, 2).reshape(128, 6 * H2)
    )
    w2_h = _kmajor((W2[256:] * _SF).astype(_BF16), K1 - 2)
    b2_col = (b2 * _SF).reshape(M2, 128).T.astype(np.float32)

    in_maps = []
    scatters = []  # per core: list of (sorted_positions, col_start)
    for c in range(G):
        sT = np.zeros((D, N), dtype=_BF16)
        img = np.zeros((128, IMG_W), dtype=_BF16)
        bs = np.zeros((128, M1 + M2 + NG), dtype=np.float32)
        bs[:, :M1] = b1[c].reshape(M1, 128)[list(_MPERM)].T
        for p_, lm_ in enumerate(_MPERM):   # relu1 logical m0-1: Relu(16*ps+16*b1)
            if lm_ < 2:
                bs[:, p_] *= _SH
        bs[:, M1 : M1 + M2] = b2_col
        sc = []
        for j, (head, pos) in enumerate(core_groups[c]):
            s0 = int(starts[j])
            if len(pos):
                sT[:, s0 : s0 + len(pos)] = state[perm[pos]].T.astype(_BF16)
                sc.append((pos, s0))
            img[:, P2 + j * K2 * A : P2 + (j + 1) * K2 * A] = (
                (W3[head] / _SF).astype(_BF16)
                .reshape(K2, 128, A).transpose(1, 0, 2).reshape(128, K2 * A)
            )
            bs[:, M1 + M2 + j] = b3[head]
        w1p = W1[c].reshape(D, M1, 128)[:, list(_MPERM), :].reshape(D, H1)
        w1_h = _kmajor(w1p.astype(_BF16), KD)    # [128, KD*H1], col k*H1+m
        for k in range(KD):
            img[:, k * PAIR : k * PAIR + c0w0] = sT[k * 128 : (k + 1) * 128, 0:c0w0]
            img[:, k * PAIR + c0w0 : (k + 1) * PAIR] = w1_h[:, k * H1 : (k + 1) * H1]
        for c0, cw in chunks[1:]:
            img[:, P0 + (c0 - c0w0) * KD : P0 + (c0 - c0w0 + cw) * KD] = (
                sT[:, c0 : c0 + cw].reshape(KD, 128, cw).transpose(1, 0, 2).reshape(128, KD * cw)
            )
        img[:, P1:P2] = w2_h
        in_maps.append({"img": img, "bs": bs, "w2q": w2q})
        scatters.append(sc)
    return nc, in_maps, scatters, (B, A)


def _run(state, idx, W1, b1, W2, b2, W3, b3, trace=False, trace_kwargs=None):
    from concourse.bass_utils import run_bass_kernel_spmd

    nc, in_maps, scatters, (B, A) = _prepare(state, idx, W1, b1, W2, b2, W3, b3)
    res = run_bass_kernel_spmd(
        nc,
        in_maps,
        core_ids=list(range(_NCORES)),
        trace=trace,
        **(trace_kwargs or {}),
    )
    out = np.zeros((B, A), dtype=np.float32)
    for c in range(len(scatters)):
        o = np.asarray(res.results[c]["out"], dtype=np.float32)  # [A, N]
        for pos, s0 in scatters[c]:
            out[pos] = o[:, s0 : s0 + len(pos)].T
    return out, res


def kernel(**inputs) -> np.ndarray:
    out, _ = _run(**inputs)
    return out

